# revision 1
# baseline (speedup 1.0000x reference)
"""DynamicSparseAttention Trainium2 kernel.

Shards B=2 x H=16 across 8 NeuronCores: core c handles batch c//4 and the
4 heads [4*(c%4), 4*(c%4)+4).  Self-contained: all shapes hardcoded.

Numerics strategy:
- importance MLP must reproduce the reference top-512 sets exactly-enough
  (min gap between 512th/513th score is ~1.9e-5).  HW fp32 matmuls get
  silently degraded when interleaved with fp32r matmuls, so the MLP uses an
  exact bf16 hi/lo split (3-term Karatsuba) with fp32 PSUM accumulation:
  error ~1e-6.
- q/k/v projections and attention run in fp32r (~1.6e-4 relative).
- binary-search for the per-head 512th-largest score uses bf16 0/1
  matmuls for cross-partition count-reduce and predicate broadcast.
"""
import os
import ml_dtypes
import numpy as np

import concourse.bass as bass
import concourse.mybir as mybir
import concourse.tile as tile
from concourse import bacc
from concourse.bass_utils import run_bass_kernel_spmd

F32 = mybir.dt.float32
BF16 = mybir.dt.bfloat16
F32R = mybir.dt.float32r
I16 = mybir.dt.int16
U8 = mybir.dt.uint8
U32 = mybir.dt.uint32
AF = mybir.ActivationFunctionType
OP = mybir.AluOpType

B, L, D = 2, 4096, 1024
H, HD, TOPK = 16, 64, 512
HIDDEN = 256
HPC = 4               # heads per core
COLS = HPC * HD       # 256 output cols per core
NG = 16               # token groups of 256
GT = 256              # tokens per group
DC = 8                # 128-row chunks of D
NITER = int(os.environ.get("KERNEL_NITER", "20"))
SLO, SHI = -2.0, 2.0  # score bounds (data range ~[-1.1, 1.1]; asserted in test)


def build_nc():
    nc = bacc.Bacc("TRN2", target_bir_lowering=False)

    th_t = nc.dram_tensor("th_t", [D, L], BF16, kind="ExternalInput")
    tl_t = nc.dram_tensor("tl_t", [D, L], BF16, kind="ExternalInput")
    wq = nc.dram_tensor("wq", [D, COLS], F32, kind="ExternalInput")
    wk = nc.dram_tensor("wk", [D, COLS], F32, kind="ExternalInput")
    wv = nc.dram_tensor("wv", [D, COLS], F32, kind="ExternalInput")
    bq = nc.dram_tensor("bq", [COLS], F32, kind="ExternalInput")
    bk = nc.dram_tensor("bk", [COLS], F32, kind="ExternalInput")
    bv = nc.dram_tensor("bv", [COLS], F32, kind="ExternalInput")
    wi1 = nc.dram_tensor("wi1", [D, HIDDEN], F32, kind="ExternalInput")
    bi1 = nc.dram_tensor("bi1", [HIDDEN], F32, kind="ExternalInput")
    wi2 = nc.dram_tensor("wi2", [HIDDEN, HPC], F32, kind="ExternalInput")
    bi2 = nc.dram_tensor("bi2", [HPC], F32, kind="ExternalInput")

    out = nc.dram_tensor("out", [L, COLS], F32, kind="ExternalOutput")
    dbg_scores = nc.dram_tensor("dbg_scores", [HPC, L], F32, kind="ExternalOutput")
    DEBUG = os.environ.get("KERNEL_DEBUG", "0") == "1"
    if DEBUG:
        dbg_idx = nc.dram_tensor("dbg_idx", [128, HPC, 32], I16, kind="ExternalOutput")
        dbg_sel = nc.dram_tensor("dbg_sel", [128, 256], F32, kind="ExternalOutput")
        dbg_idx4 = nc.dram_tensor("dbg_idx4", [16, HPC, 32], F32, kind="ExternalOutput")
        dbg_hid = nc.dram_tensor("dbg_hid", [128, 2, L], F32, kind="ExternalOutput")
        dbg_wih = nc.dram_tensor("dbg_wih", [128, DC, HIDDEN], F32, kind="ExternalOutput")
        dbg_wil = nc.dram_tensor("dbg_wil", [128, DC, HIDDEN], F32, kind="ExternalOutput")
        dbg_tth = nc.dram_tensor("dbg_tth", [128, DC, GT], F32, kind="ExternalOutput")
        dbg_ttl = nc.dram_tensor("dbg_ttl", [128, DC, GT], F32, kind="ExternalOutput")

    k_dram = nc.dram_tensor("k_scratch", [L, COLS], F32, kind="Internal")
    v_dram = nc.dram_tensor("v_scratch", [L, COLS], F32, kind="Internal")
    s_scratch = nc.dram_tensor("s_scratch", [HPC, L], F32, kind="Internal")

    # ---- constants, embedded in NEFF ----
    ident_np = np.eye(128, dtype=np.float32)
    # head h owns partitions [32h, 32h+16); token l = (p%32) + 16*ff
    pp = np.arange(128) % 32
    iota1_np = (np.where(pp < 16, pp, -10**9)[:, None] + 16 * np.arange(256)[None, :]
                + 1).astype(np.float32)
    g64_np = ((np.arange(128)[:, None] // 32 == np.arange(4)[None, :])
              & (pp[:, None] < 16)).astype(np.float32)          # [128, 4]
    b4_np = (np.arange(128)[None, :] // 32 == np.arange(4)[:, None]).astype(np.float32)
    m128_np = ((np.arange(128)[:, None] // 32 == np.arange(128)[None, :] // 32)
               & (pp[:, None] < 16)).astype(np.float32)      # [128(q), 128(p)]
    r16_np = (np.arange(16)[:, None] == (np.arange(128)[None, :] % 16)).astype(np.float32)
    ones_np = np.ones((1, 128), dtype=np.float32)

    ident_d = nc.inline_tensor(ident_np, name="ident128")
    iota1_d = nc.inline_tensor(iota1_np, name="iota1")
    g64_d = nc.inline_tensor(g64_np, name="g64")
    m128_d = nc.inline_tensor(m128_np, name="m128")
    r16_d = nc.inline_tensor(r16_np, name="r16")
    ones_d = nc.inline_tensor(ones_np, name="ones1")

    _prev_pe = [None]

    def _chain(r):
        _prev_pe[0] = r.ins
        return r

    def mm(*a, **kw):
        return _chain(nc.tensor.matmul(*a, **kw))

    def tr(*a, **kw):
        return _chain(nc.tensor.transpose(*a, **kw))

    with tile.TileContext(nc) as tc:
        with (
            tc.tile_pool(name="persist", bufs=1) as pp_,
            tc.tile_pool(name="wts", bufs=1) as pw,
            tc.tile_pool(name="tokp", bufs=2) as ptok,
            tc.tile_pool(name="toktp", bufs=2) as ptokt,
            tc.tile_pool(name="kvsb", bufs=2) as pkv,
            tc.tile_pool(name="small", bufs=2) as psm,
            tc.tile_pool(name="psA", bufs=3, space="PSUM") as psA,
            tc.tile_pool(name="psB", bufs=3, space="PSUM") as psB,
        ):
            # ---------- constants ----------
            ident = pp_.tile([128, 128], F32)
            nc.sync.dma_start(ident, ident_d[:, :])
            iota1 = pp_.tile([128, 256], F32)
            nc.sync.dma_start(iota1, iota1_d[:, :])
            m128f = pp_.tile([128, 128], F32)
            nc.sync.dma_start(m128f, m128_d[:, :])
            m128 = pp_.tile([128, 128], BF16)
            nc.vector.tensor_copy(m128, m128f)
            r16 = pp_.tile([16, 128], F32)
            nc.sync.dma_start(r16, r16_d[:, :])
            ones1f = pp_.tile([1, 128], F32)
            nc.sync.dma_start(ones1f, ones_d[:, :])
            ones1 = pp_.tile([1, 128], F32R)
            nc.vector.tensor_copy(ones1, ones1f)

            # ---------- weights ----------
            wtmp = pw.tile([128, DC, COLS], F32, tag="shE", bufs=2, name="wtmp")
            nc.sync.dma_start(wtmp, wq.rearrange("(c p) n -> p c n", p=128))
            wqs = pw.tile([128, DC, COLS], F32R)
            nc.vector.tensor_copy(wqs, wtmp)
            wtmp2 = pw.tile([128, DC, COLS], F32, tag="shE", bufs=2, name="wtmp2")
            nc.sync.dma_start(wtmp2, wk.rearrange("(c p) n -> p c n", p=128))
            wks = pw.tile([128, DC, COLS], F32R)
            nc.vector.tensor_copy(wks, wtmp2)
            wtmp3 = pw.tile([128, DC, COLS], F32, tag="shE", bufs=2, name="wtmp3")
            nc.sync.dma_start(wtmp3, wv.rearrange("(c p) n -> p c n", p=128))
            wvs = pw.tile([128, DC, COLS], F32R)
            nc.vector.tensor_copy(wvs, wtmp3)

            wi1f = pw.tile([128, DC, HIDDEN], F32, tag="shE", bufs=2)
            nc.sync.dma_start(wi1f, wi1.rearrange("(c p) n -> p c n", p=128))
            wi1h = pw.tile([128, DC, HIDDEN], BF16)
            nc.vector.tensor_copy(wi1h, wi1f)
            wi1l = pw.tile([128, DC, HIDDEN], BF16)
            nc.vector.tensor_sub(wi1l, wi1f, wi1h)

            if DEBUG:
                nc.gpsimd.dma_start(dbg_wih[:, :, :], wi1h)
                nc.gpsimd.dma_start(dbg_wil[:, :, :], wi1l)
            wi2f = pw.tile([128, 2, HPC], F32)
            nc.sync.dma_start(wi2f, wi2.rearrange("(c p) n -> p c n", p=128))
            wi2h = pw.tile([128, 2, HPC], BF16)
            nc.vector.tensor_copy(wi2h, wi2f)
            wi2l = pw.tile([128, 2, HPC], BF16)
            nc.vector.tensor_sub(wi2l, wi2f, wi2h)

            bqs = pw.tile([128, 2], F32)
            nc.sync.dma_start(bqs, bq.rearrange("(c p) -> p c", p=128))
            bi1s = pw.tile([128, 2], F32)
            nc.sync.dma_start(bi1s, bi1.rearrange("(c p) -> p c", p=128))
            bkf = pw.tile([1, COLS], F32)
            nc.sync.dma_start(bkf, bk[None, :])
            bks = pw.tile([1, COLS], F32R)
            nc.vector.tensor_copy(bks, bkf)
            bvf = pw.tile([1, COLS], F32)
            nc.sync.dma_start(bvf, bv[None, :])
            bvs = pw.tile([1, COLS], F32R)
            nc.vector.tensor_copy(bvs, bvf)
            bi2s = pw.tile([HPC, 1], F32)
            nc.sync.dma_start(bi2s, bi2[:, None])

            # ---------- persistent big tensors ----------
            # qT pair tiles: partitions = [head 2p (hd 0..63); head 2p+1]
            qT = [pp_.tile([128, L], F32R, tag=f"qT{p2}", name=f"qT{p2}") for p2 in range(2)]
            # gelu-hi lives in the slots later reused as the output buffers
            ghi = [pp_.tile([128, L], BF16, tag=f"big{t}", name=f"ghi{t}")
                   for t in range(2)]

            # ---------- stage 1 ----------
            glo_w = None
            for g in range(NG):
                if g % 2 == 0:
                    glo_w = pw.tile([128, 2, 512], BF16, tag="gloW", bufs=2,
                                    name="glo_w")
                sl_g = slice(g * GT, (g + 1) * GT)
                tTh = ptokt.tile([128, DC, GT], BF16, tag="tTh", name="tTh")
                nc.sync.dma_start(tTh, th_t[:, sl_g].rearrange("(c p) t -> p c t", p=128))
                tTl = ptokt.tile([128, DC, GT], BF16, tag="tTl", name="tTl")
                nc.sync.dma_start(tTl, tl_t[:, sl_g].rearrange("(c p) t -> p c t", p=128))
                tTr = ptokt.tile([128, DC, GT], F32R, tag="tTr", name="tTr", bufs=2)
                nc.vector.tensor_add(tTr, tTh, tTl)

                # importance MLP hidden via exact bf16 3-term split -> gelu hi/lo
                for ht in range(2):
                    hps = psB.tile([128, GT], F32, tag="psM", bufs=2)
                    n3 = 3 * DC
                    i = 0
                    for dc in range(DC):
                        for (a, b_) in ((tTh, wi1h), (tTl, wi1h), (tTh, wi1l)):
                            mm(
                                hps, b_[:, dc, ht * 128:(ht + 1) * 128], a[:, dc, :],
                                start=(i == 0), stop=(i == n3 - 1),
                            )
                            i += 1
                    gtmp = psm.tile([128, GT], F32, tag="gtmp", bufs=1)
                    nc.scalar.activation(
                        gtmp, hps, AF.Gelu, bias=bi1s[:, ht:ht + 1], scale=1.0)
                    if DEBUG:
                        nc.sync.dma_start(
                            dbg_hid[:, ht, g * GT:(g + 1) * GT], gtmp)
                    dsth = ghi[ht][:, g * GT:(g + 1) * GT]
                    dstl = glo_w[:, ht, (g % 2) * GT:(g % 2) * GT + GT]
                    nc.vector.tensor_copy(dsth, gtmp)
                    nc.vector.tensor_sub(dstl, gtmp, dsth)

                # MLP2 chunk when a 512-col slab of gelu is ready (bf16 3-term)
                if g % 2 == 1:
                    c8 = g // 2
                    ips = psB.tile([HPC, 512], F32, tag="psM", bufs=2)
                    sl = slice(c8 * 512, (c8 + 1) * 512)
                    i = 0
                    for kc in range(2):
                        for (a, b_) in ((ghi[kc][:, sl], wi2h),
                                        (glo_w[:, kc, :], wi2h),
                                        (ghi[kc][:, sl], wi2l)):
                            mm(
                                ips, b_[:, kc, :], a,
                                start=(i == 0), stop=(i == 5),
                            )
                            i += 1
                    imp_c = psm.tile([HPC, 512], F32, tag="imp_c", bufs=1)
                    nc.vector.tensor_scalar_add(imp_c, ips, bi2s)
                    nc.sync.dma_start(s_scratch[:, sl], imp_c)
                    nc.sync.dma_start(dbg_scores[:, sl], imp_c)

                # qT (transposed orientation, fp32r)
                for p2 in range(2):
                    qps = psA.tile([128, GT], F32, tag="psA")
                    for dc in range(DC):
                        mm(
                            qps, wqs[:, dc, p2 * 128:(p2 + 1) * 128], tTr[:, dc, :],
                            start=(dc == 0), stop=(dc == DC - 1),
                        )
                    nc.vector.tensor_scalar_add(
                        qT[p2][:, g * GT:(g + 1) * GT], qps, bqs[:, p2:p2 + 1]
                    )

                # k, v (normal orientation, fp32r) -> DRAM scratch
                for (ws, brow, dram) in ((wks, bks, k_dram), (wvs, bvs, v_dram)):
                    kv = pkv.tile([128, 2, COLS], F32)
                    for s in range(2):
                        kps = psA.tile([128, COLS], F32, tag="psA")
                        for dc in range(DC):
                            mm(
                                kps, tTr[:, dc, s * 128:(s + 1) * 128], ws[:, dc, :],
                                start=(dc == 0), stop=False,
                            )
                        mm(kps, ones1, brow, start=False, stop=True)
                        nc.scalar.copy(kv[:, s, :], kps)
                    nc.sync.dma_start(
                        dram[g * GT:(g + 1) * GT, :].rearrange("(s p) n -> p s n", p=128),
                        kv)

            # ---------- stage 2/3: scores -> sc2, top-k threshold ----------
            sc2 = pp_.tile([128, 256], F32)
            nc.vector.memset(sc2, 0.0)
            for h in range(HPC):
                nc.sync.dma_start(
                    sc2[32 * h:32 * h + 16, :],
                    s_scratch[h, :].rearrange("(ff pp) -> pp ff", pp=16),
                )

            lo = pp_.tile([128, 1], F32)
            hi = pp_.tile([128, 1], F32)
            mid = pp_.tile([128, 1], F32)
            cond = pp_.tile([128, 1], U8)
            ncond = pp_.tile([128, 1], U8)
            cnt = pp_.tile([128, 1], BF16)
            ge = pp_.tile([128, 256], F32, tag="gesel")
            nc.vector.memset(lo, SLO)
            nc.vector.memset(hi, SHI)
            for _ in range(NITER):
                nc.vector.tensor_add(mid, lo, hi)
                nc.vector.tensor_scalar_mul(mid, mid, 0.5)
                nc.vector.tensor_scalar(
                    ge, sc2, mid, None, op0=OP.is_gt, op1=OP.add, accum_out=cnt
                )
                cb = psB.tile([128, 1], F32, tag="psB")
                mm(cb, m128, cnt, start=True, stop=True)
                nc.vector.tensor_scalar(cond, cb, float(TOPK) - 0.5, None, op0=OP.is_gt)
                nc.vector.tensor_scalar(ncond, cb, float(TOPK) - 0.5, None, op0=OP.is_le)
                nc.vector.copy_predicated(lo, cond, mid)
                nc.vector.copy_predicated(hi, ncond, mid)

            # selected-index values (l or -1); threshold lo is per-partition
            sel = pp_.tile([128, 256], F32, tag="gesel", name="sel")
            nc.vector.tensor_scalar(sel, sc2, lo, None, op0=OP.is_gt)
            nc.vector.tensor_mul(sel, sel, iota1)
            nc.vector.tensor_scalar_sub(sel, sel, 1.0)

            nfound = pp_.tile([16, HPC], U32)
            idx4 = pp_.tile([16, HPC, 32], F32)
            for h in range(HPC):
                selh = psm.tile([16, 256], F32, tag="selh", bufs=1)
                nc.sync.dma_start(selh, sel[32 * h:32 * h + 16, :])
                idxfh = psm.tile([16, 32], F32, tag="idxfh")
                nc.gpsimd.sparse_gather(
                    idxfh, selh, num_found=nfound[0:1, h:h + 1],
                )
                nc.sync.dma_start(idx4[:, h, :], idxfh)
            rp = psB.tile([128, 128], F32, tag="psB")
            mm(rp, r16, idx4.rearrange("p h w -> p (h w)"), start=True, stop=True)
            idx16 = pp_.tile([128, HPC, 32], I16)
            nc.vector.tensor_copy(idx16.rearrange("p h w -> p (h w)"), rp)
            if DEBUG:
                nc.sync.dma_start(dbg_idx[:, :, :], idx16)
                nc.sync.dma_start(dbg_sel[:, :], sel)
                nc.sync.dma_start(dbg_idx4[:, :, :], idx4)

            # ---------- stage 4: gather + attention per head ----------
            onescol = pp_.tile([128, 4], F32)
            nc.vector.memset(onescol, 1.0)
            obuf = [pp_.tile([128, 16, COLS], F32, tag=f"big{t}", name=f"obuf{t}")
                    for t in range(2)]
            NHEADS = int(os.environ.get("KERNEL_NHEADS", str(HPC)))
            for hp in range(1):
                pair = tuple(range(NHEADS))
                kselTs, vselAs = {}, {}
                for h in pair:
                    h2 = h % 2
                    ksel = psm.tile([128, 4, HD], F32, tag="ksel")
                    vselA = psm.tile([128, 4, HD + 1], F32R, tag="vsel", bufs=4)
                    vsel = psm.tile([128, 4, HD], F32, tag="vsel0", bufs=2)
                    nc.gpsimd.dma_gather(
                        ksel, k_dram[:, HD * h:HD * (h + 1)], idx16[:, h, :],
                        num_idxs=TOPK, num_idxs_reg=TOPK, elem_size=HD, elem_step=COLS,
                    )
                    nc.gpsimd.dma_gather(
                        vsel, v_dram[:, HD * h:HD * (h + 1)], idx16[:, h, :],
                        num_idxs=TOPK, num_idxs_reg=TOPK, elem_size=HD, elem_step=COLS,
                    )
                    nc.vector.tensor_copy(vselA[:, :, HD:HD + 1], onescol[:, :, None])
                    nc.vector.tensor_copy(vselA[:, :, 0:HD], vsel)
                    vselAs[h] = vselA

                    kselT = psm.tile([128, TOPK], F32R, tag="kselT", bufs=4)
                    if h2 == 0:
                        for kt in range(4):
                            tp = psB.tile([128, 128], F32, tag="psB")
                            tr(tp[:64, :], ksel[:, kt, :], ident)
                            nc.vector.tensor_copy(
                                kselT[:64, kt * 128:(kt + 1) * 128], tp[:64, :]
                            )
                    else:
                        # matmul PSUM out must start at partition 0; build at
                        # base 0 then partition-shift via SBUF->SBUF DMA.
                        ktmp = psm.tile([64, TOPK], F32R, tag="ktmp", bufs=2)
                        for kt in range(4):
                            tp = psB.tile([128, 128], F32, tag="psB")
                            tr(tp[:64, :], ksel[:, kt, :], ident)
                            nc.vector.tensor_copy(
                                ktmp[:, kt * 128:(kt + 1) * 128], tp[:64, :]
                            )
                        nc.sync.dma_start(kselT[64:128, :], ktmp)
                    kselTs[h] = kselT

                for qc in range(8):
                    for h in pair:
                        p2, h2 = h // 2, h % 2
                        base = 64 * h2
                        kselT, vselA = kselTs[h], vselAs[h]
                        expT = pw.tile([128, 4, 512], F32R, tag="shE", bufs=2,
                                       name="expT")
                        for kt in range(4):
                            lp = psA.tile([128, 512], F32, tag="psA")
                            mm(
                                lp,
                                kselT[base:base + 64, kt * 128:(kt + 1) * 128],
                                qT[p2][base:base + 64, qc * 512:(qc + 1) * 512],
                                start=True, stop=True,
                            )
                            nc.scalar.activation(expT[:, kt, :], lp, AF.Exp,
                                                 scale=0.125)
                        avp = psB.tile([HD + 1, 512], F32, tag="psB")
                        for kt in range(4):
                            mm(
                                avp, vselA[:, kt, :], expT[:, kt, :],
                                start=(kt == 0), stop=(kt == 3),
                            )
                        av = psm.tile([HD + 1, 512], F32, tag="av", bufs=2)
                        nc.vector.tensor_copy(av, avp)
                        for qs in range(4):
                            qt = qc * 4 + qs
                            tp2 = psB.tile([128, HD + 1], F32, tag="psB")
                            tr(
                                tp2, av[:, qs * 128:(qs + 1) * 128],
                                ident[:HD + 1, :HD + 1]
                            )
                            rcp = psm.tile([128, 1], F32, tag="rcp")
                            nc.vector.reciprocal(rcp, tp2[:, HD:HD + 1])
                            ob = obuf[qt // 16]
                            nc.vector.tensor_scalar_mul(
                                ob[:, qt % 16, HD * h:HD * (h + 1)], tp2[:, :HD], rcp
                            )

            # ---------- stage 5: output ----------
            for q4 in range(8):
                qt = q4 * 4
                nc.sync.dma_start(
                    out[qt * 128:(qt + 4) * 128, :].rearrange(
                        "(q p) n -> p q n", p=128),
                    obuf[qt // 16][:, qt % 16:qt % 16 + 4, :],
                )

    nc.compile()
    return nc


_NC = None


def _get_nc():
    global _NC
    if _NC is None:
        _NC = build_nc()
    return _NC


def make_in_maps(**inputs):
    t = {k: np.ascontiguousarray(np.asarray(v, dtype=np.float32)) for k, v in inputs.items()}
    in_maps = []
    for c in range(8):
        b, hg = c // 4, c % 4
        cs = COLS * hg
        hs = HPC * hg
        tok = t["tokens"][b]
        th = tok.astype(ml_dtypes.bfloat16)
        tl = (tok - th.astype(np.float32)).astype(ml_dtypes.bfloat16)
        in_maps.append({
            "th_t": np.ascontiguousarray(th.T),
            "tl_t": np.ascontiguousarray(tl.T),
            "wq": np.ascontiguousarray(t["Wq"][:, cs:cs + COLS]),
            "bq": np.ascontiguousarray(t["bq"][cs:cs + COLS]),
            "wk": np.ascontiguousarray(t["Wk"][:, cs:cs + COLS]),
            "bk": np.ascontiguousarray(t["bk"][cs:cs + COLS]),
            "wv": np.ascontiguousarray(t["Wv"][:, cs:cs + COLS]),
            "bv": np.ascontiguousarray(t["bv"][cs:cs + COLS]),
            "wi1": t["Wi1"],
            "bi1": t["bi1"],
            "wi2": np.ascontiguousarray(t["Wi2"][:, hs:hs + HPC]),
            "bi2": np.ascontiguousarray(t["bi2"][hs:hs + HPC]),
        })
    return in_maps


def kernel(**inputs) -> np.ndarray:
    nc = _get_nc()
    in_maps = make_in_maps(**inputs)
    res = run_bass_kernel_spmd(nc, in_maps, core_ids=list(range(8)))
    out = np.empty((B, L, D), dtype=np.float32)
    for c in range(8):
        b, hg = c // 4, c % 4
        out[b, :, COLS * hg:COLS * (hg + 1)] = res.results[c]["out"]
    return out



# revision 6
# speedup vs baseline: 1.2109x; 1.2109x over previous
"""DynamicSparseAttention Trainium2 kernel (v2).

Shards B=2 x H=16 across 8 NeuronCores: core c handles batch c//4 and the
4 heads [4*(c%4), 4*(c%4)+4).  Self-contained: all shapes hardcoded.

v2 design vs v1:
- importance MLP deduplicated across the 4 same-batch cores: each core
  computes the exact bf16 3-term MLP on its 1024-token quarter for ALL 16
  heads, then an AllGather over the 4-core replica group + an index-driven
  dma_gather row-pick (per-core head selection via an int16 input tensor,
  keeping the program SPMD-identical) recovers this core's 4 heads x 4096
  token scores.
- the 20-iteration binary search for the 512th score is replaced by one
  gpsimd kth_largest per head (exact 512th-largest via desc[k_adj+1]).
- projections restructured: fused k|v weight (one N=512 matmul stream,
  token-stationary), q projected transposed (weight-stationary), both in
  fp32r; top-k/gather chain overlaps the projection matmuls.
- attention: exp fused over PSUM bank pairs, weight-stationary where PSUM
  allows; same gather/attention dataflow as v1 otherwise.
"""
import os
import ml_dtypes
import numpy as np

import concourse.bass as bass
import concourse.mybir as mybir
import concourse.tile as tile
from concourse import bacc
from concourse.bass_utils import run_bass_kernel_spmd

F32 = mybir.dt.float32
BF16 = mybir.dt.bfloat16
F32R = mybir.dt.float32r
I16 = mybir.dt.int16
U32 = mybir.dt.uint32
AF = mybir.ActivationFunctionType
OP = mybir.AluOpType

B, L, D = 2, 4096, 1024
H, HD, TOPK = 16, 64, 512
HIDDEN = 256
HPC = 4               # heads per core
COLS = HPC * HD       # 256 q cols per core
KVC = 2 * COLS        # fused k|v cols
DC = 8                # 128-row chunks of D
LLOC = L // 4         # local MLP token quarter
GB = 512              # stage-B token group
NGB = L // GB


def build_nc():
    nc = bacc.Bacc("TRN2", target_bir_lowering=False)

    tok_t = nc.dram_tensor("tok_t", [D, L], F32R, kind="ExternalInput")
    mth_t = nc.dram_tensor("mth_t", [D, LLOC], BF16, kind="ExternalInput")
    mtl_t = nc.dram_tensor("mtl_t", [D, LLOC], BF16, kind="ExternalInput")
    wq = nc.dram_tensor("wq", [D, COLS], F32R, kind="ExternalInput")
    bq = nc.dram_tensor("bq", [COLS], F32, kind="ExternalInput")
    wkv = nc.dram_tensor("wkv", [D, KVC], F32R, kind="ExternalInput")
    bkv = nc.dram_tensor("bkv", [1, KVC], F32R, kind="ExternalInput")
    wi1h = nc.dram_tensor("wi1h", [D, HIDDEN], BF16, kind="ExternalInput")
    wi1l = nc.dram_tensor("wi1l", [D, HIDDEN], BF16, kind="ExternalInput")
    bi1 = nc.dram_tensor("bi1", [HIDDEN], F32, kind="ExternalInput")
    wi2h = nc.dram_tensor("wi2h", [HIDDEN, H], BF16, kind="ExternalInput")
    wi2l = nc.dram_tensor("wi2l", [HIDDEN, H], BF16, kind="ExternalInput")
    bi2 = nc.dram_tensor("bi2", [H], F32, kind="ExternalInput")
    hrows = nc.dram_tensor("hrows", [128, 1], I16, kind="ExternalInput")

    out = nc.dram_tensor("out", [L, COLS], F32, kind="ExternalOutput")
    dbg_scores = nc.dram_tensor("dbg_scores", [HPC, L], F32, kind="ExternalOutput")

    kv_dram = nc.dram_tensor("kv_scratch", [L, KVC], F32, kind="Internal")

    # ---- constants, embedded in NEFF ----
    ident_np = np.eye(128, dtype=np.float32)
    # head h owns partitions [32h, 32h+16); token l = (p%32) + 16*ff
    pp = np.arange(128) % 32
    iota1_np = (np.where(pp < 16, pp, -10**9)[:, None] + 16 * np.arange(256)[None, :]
                + 1).astype(np.float32)
    b4_np = (np.arange(128)[None, :] // 32 == np.arange(4)[:, None]).astype(np.float32)
    r16_np = (np.arange(16)[:, None] == (np.arange(128)[None, :] % 16)).astype(np.float32)
    ones_np = np.ones((1, 128), dtype=np.float32)

    ident_d = nc.inline_tensor(ident_np, name="ident128")
    iota1_d = nc.inline_tensor(iota1_np, name="iota1")
    b4_d = nc.inline_tensor(b4_np, name="b4")
    r16_d = nc.inline_tensor(r16_np, name="r16")
    ones_d = nc.inline_tensor(ones_np, name="ones1")

    def mm(*a, **kw):
        return nc.tensor.matmul(*a, **kw)

    def tr(*a, **kw):
        return nc.tensor.transpose(*a, **kw)

    with tile.TileContext(nc) as tc:
        with (
            tc.tile_pool(name="persist", bufs=1) as pp_,
            tc.tile_pool(name="wts", bufs=1) as pw,
            tc.tile_pool(name="tokp", bufs=2) as ptok,
            tc.tile_pool(name="kvsb", bufs=2) as pkv,
            tc.tile_pool(name="small", bufs=2) as psm,
            tc.tile_pool(name="psP", bufs=2, space="PSUM") as psP,
            tc.tile_pool(name="dram", bufs=1, space="DRAM") as pdram,
        ):
            # ---------- constants ----------
            ident = pp_.tile([128, 128], F32)
            nc.sync.dma_start(ident, ident_d[:, :])
            iota1 = pp_.tile([128, 256], F32)
            nc.sync.dma_start(iota1, iota1_d[:, :])
            b4 = pp_.tile([4, 128], F32)
            nc.sync.dma_start(b4, b4_d[:, :])
            r16 = pp_.tile([16, 128], F32)
            nc.sync.dma_start(r16, r16_d[:, :])
            ones1f = pp_.tile([1, 128], F32)
            nc.sync.dma_start(ones1f, ones_d[:, :])
            ones1 = pp_.tile([1, 128], F32R)
            nc.vector.tensor_copy(ones1, ones1f)

            # ---------- weights (direct dtype loads, no conversions) ----------
            wqs = pw.tile([128, DC, COLS], F32R)
            nc.sync.dma_start(wqs, wq.rearrange("(c p) n -> p c n", p=128))
            wkvs = pw.tile([128, DC, KVC], F32R)
            nc.sync.dma_start(wkvs, wkv.rearrange("(c p) n -> p c n", p=128))
            bkvr = pw.tile([1, KVC], F32R)
            nc.sync.dma_start(bkvr, bkv[:, :])
            wi1hs = pw.tile([128, DC, HIDDEN], BF16)
            nc.sync.dma_start(wi1hs, wi1h.rearrange("(c p) n -> p c n", p=128))
            wi1ls = pw.tile([128, DC, HIDDEN], BF16)
            nc.sync.dma_start(wi1ls, wi1l.rearrange("(c p) n -> p c n", p=128))
            wi2hs = pw.tile([128, 2, H], BF16)
            nc.sync.dma_start(wi2hs, wi2h.rearrange("(c p) n -> p c n", p=128))
            wi2ls = pw.tile([128, 2, H], BF16)
            nc.sync.dma_start(wi2ls, wi2l.rearrange("(c p) n -> p c n", p=128))
            bi1s = pw.tile([128, 2], F32)
            nc.sync.dma_start(bi1s, bi1.rearrange("(c p) -> p c", p=128))
            bi2s = pw.tile([H, 1], F32)
            nc.sync.dma_start(bi2s, bi2[:, None])
            bqs2 = pw.tile([128, 2], F32)
            nc.sync.dma_start(bqs2, bq.rearrange("(c p) -> p c", p=128))
            hidx = pw.tile([128, 1], I16)
            nc.sync.dma_start(hidx, hrows[:, :])

            # MLP-local tokens (obuf reuses these slots later via tags)
            mth = pp_.tile([128, DC, LLOC], BF16, tag="bigA", name="mth")
            nc.sync.dma_start(mth, mth_t.rearrange("(c p) t -> p c t", p=128))
            mtl = pp_.tile([128, DC, LLOC], BF16, tag="bigB", name="mtl")
            nc.sync.dma_start(mtl, mtl_t.rearrange("(c p) t -> p c t", p=128))

            # dram bounce buffers (pool tiles => dependency-tracked)
            imp_st = pdram.tile([H, LLOC], F32)
            imp_all = pdram.tile([4, H, LLOC], F32)
            sc_d = pdram.tile([H, LLOC], F32)

            # persistent big tensors
            qT = [pp_.tile([128, L], F32R, tag=f"qT{p2}", name=f"qT{p2}")
                  for p2 in range(2)]
            ghi = pp_.tile([128, 2, LLOC], BF16, name="ghi")
            glo = pp_.tile([128, 2, LLOC], BF16, name="glo")

            # ---------- stage A: importance MLP on local quarter ----------
            hps = [psP.tile([128, 2, GB], F32, tag="P2x2", name=f"hps{i}")
                   for i in range(2)]
            for ht in range(2):
                hsl = slice(ht * 128, (ht + 1) * 128)
                for dc in range(DC):
                    for (a_, w_) in ((mth, wi1hs), (mtl, wi1hs), (mth, wi1ls)):
                        first = dc == 0 and w_ is wi1hs and a_ is mth
                        last = dc == DC - 1 and w_ is wi1ls
                        for g in range(2):
                            mm(
                                hps[ht][:, g, :], w_[:, dc, hsl],
                                a_[:, dc, g * GB:(g + 1) * GB],
                                start=first, stop=last,
                            )
            for ht in range(2):
                for g in range(2):
                    gtmp = psm.tile([128, GB], F32, tag="gtmp", bufs=2)
                    nc.scalar.activation(
                        gtmp, hps[ht][:, g, :], AF.Gelu,
                        bias=bi1s[:, ht:ht + 1], scale=1.0)
                    dsth = ghi[:, ht, g * GB:(g + 1) * GB]
                    dstl = glo[:, ht, g * GB:(g + 1) * GB]
                    nc.vector.tensor_copy(dsth, gtmp)
                    nc.vector.tensor_sub(dstl, gtmp, dsth)

            ips = [psP.tile([H, GB], F32, tag="P1", name=f"ips{g}")
                   for g in range(2)]
            for hc in range(2):
                for (a_, w_) in ((ghi, wi2hs), (glo, wi2hs), (ghi, wi2ls)):
                    first = hc == 0 and w_ is wi2hs and a_ is ghi
                    last = hc == 1 and w_ is wi2ls
                    for g in range(2):
                        mm(
                            ips[g], w_[:, hc, :],
                            a_[:, hc, g * GB:(g + 1) * GB],
                            start=first, stop=last,
                        )
            impsb = psm.tile([H, 2, GB], F32, tag="impsb", bufs=1)
            for g in range(2):
                nc.vector.tensor_scalar_add(impsb[:, g, :], ips[g], bi2s)
            nc.sync.dma_start(imp_st[:], impsb.rearrange("p a b -> p (a b)"))

            # ---------- stage A2: exchange + per-head top-512 threshold ----
            nc.gpsimd.collective_compute(
                "AllGather",
                mybir.AluOpType.bypass,
                replica_groups=[[0, 1, 2, 3], [4, 5, 6, 7]],
                ins=[imp_st.opt()],
                outs=[imp_all.opt()],
            )
            sc = pp_.tile([128, 1, LLOC], F32, name="sc")
            nc.gpsimd.dma_gather(
                sc, imp_all[:].rearrange("a b c -> (a b) c"), hidx,
                num_idxs=16, num_idxs_reg=16, elem_size=LLOC, elem_step=LLOC,
            )
            nc.gpsimd.dma_start(sc_d[:], sc[0:16, 0, :])
            nc.gpsimd.dma_start(
                dbg_scores.rearrange("h (q j) -> (h q) j", q=4), sc[0:16, 0, :])

            scks = [pp_.tile([128, 32], F32, name=f"sck{h}") for h in range(4)]
            thrs = [pp_.tile([1, 2], F32, name=f"thr{h}") for h in range(4)]
            thr4 = pp_.tile([4, 1], F32)
            for h in range(4):
                nc.gpsimd.dma_start(
                    scks[h],
                    sc_d[4 * h:4 * h + 4, :].rearrange("q (a b) -> (q a) b", a=32),
                )
                nc.gpsimd.kth_largest(
                    thrs[h][:, :], scks[h][:, :], n_per_lane=32, k=510,
                    quantile=1.0 - 510.5 / 4095.0,
                )
                nc.gpsimd.dma_start(thr4[h:h + 1, 0:1], thrs[h][0:1, 1:2])
            thrp = psP.tile([128, 1], F32, tag="PS")
            mm(thrp, b4, thr4, start=True, stop=True)
            thrb = pp_.tile([128, 1], F32)
            nc.vector.tensor_copy(thrb, thrp)

            sc2 = pp_.tile([128, 256], F32, name="sc2")
            for h in range(4):
                nc.gpsimd.dma_start(
                    sc2[32 * h:32 * h + 16, :],
                    sc_d[4 * h:4 * h + 4, :].rearrange("q (f p) -> p (q f)", p=16),
                )
            sel = pp_.tile([128, 256], F32, name="sel")
            nc.vector.tensor_scalar(sel, sc2, thrb, None, op0=OP.is_ge)
            nc.vector.tensor_mul(sel, sel, iota1)
            nc.vector.tensor_scalar_sub(sel, sel, 1.0)

            nfound = pp_.tile([16, HPC], U32)
            idx4 = pp_.tile([16, HPC, 32], F32)
            for h in range(HPC):
                selh = psm.tile([16, 256], F32, tag="selh", bufs=1)
                nc.sync.dma_start(selh, sel[32 * h:32 * h + 16, :])
                idxfh = psm.tile([16, 32], F32, tag="idxfh")
                nc.gpsimd.sparse_gather(
                    idxfh, selh, num_found=nfound[0:1, h:h + 1],
                )
                nc.sync.dma_start(idx4[:, h, :], idxfh)
            rp = psP.tile([128, 128], F32, tag="PS")
            mm(rp, r16, idx4.rearrange("p h w -> p (h w)"), start=True, stop=True)
            idx16 = pp_.tile([128, HPC, 32], I16)
            nc.vector.tensor_copy(idx16.rearrange("p h w -> p (h w)"), rp)

            # ---------- stage B: projections (kv token-major, q transposed) --
            for g in range(NGB):
                sl_g = slice(g * GB, (g + 1) * GB)
                tokT = ptok.tile([128, DC, GB], F32R, tag="tokT", name="tokT")
                nc.sync.dma_start(
                    tokT, tok_t[:, sl_g].rearrange("(c p) t -> p c t", p=128))

                for sbh in range(2):
                    kvsb = pkv.tile([128, 2, KVC], F32, tag="kvsb", name="kvsb")
                    kvps = psP.tile([128, 2, KVC], F32, tag="P2x2")
                    for sj in range(2):
                        sb = 2 * sbh + sj
                        for dc in range(DC):
                            mm(
                                kvps[:, sj, :],
                                tokT[:, dc, sb * 128:(sb + 1) * 128],
                                wkvs[:, dc, :],
                                start=(dc == 0), stop=False,
                            )
                        mm(kvps[:, sj, :], ones1, bkvr, start=False, stop=True)
                        nc.vector.tensor_copy(kvsb[:, sj, :], kvps[:, sj, :])
                    nc.sync.dma_start(
                        kv_dram[g * GB + sbh * 256:g * GB + sbh * 256 + 256, :]
                        .rearrange("(s p) n -> p s n", p=128), kvsb)

                for p2 in range(2):
                    qps = psP.tile([128, GB], F32, tag="P1")
                    for dc in range(DC):
                        mm(
                            qps, wqs[:, dc, p2 * 128:(p2 + 1) * 128],
                            tokT[:, dc, :],
                            start=(dc == 0), stop=(dc == DC - 1),
                        )
                    nc.vector.tensor_scalar_add(
                        qT[p2][:, sl_g], qps, bqs2[:, p2:p2 + 1]
                    )

            # ---------- stage C: gather + attention per head ----------
            onescol = pp_.tile([128, 4], F32)
            nc.vector.memset(onescol, 1.0)
            obuf = [pp_.tile([128, 16, COLS], F32, tag=t, name=f"obuf{t}")
                    for t in ("bigA", "bigB")]
            kselTs, vselAs = {}, {}
            for h in range(HPC):
                h2 = h % 2
                ksel = psm.tile([128, 4, HD], F32, tag="ksel")
                vselA = psm.tile([128, 4, HD + 1], F32R, tag="vsel", bufs=4)
                vsel = psm.tile([128, 4, HD], F32, tag="vsel0", bufs=2)
                nc.gpsimd.dma_gather(
                    ksel, kv_dram[:, HD * h:HD * (h + 1)], idx16[:, h, :],
                    num_idxs=TOPK, num_idxs_reg=TOPK, elem_size=HD, elem_step=KVC,
                )
                nc.gpsimd.dma_gather(
                    vsel, kv_dram[:, COLS + HD * h:COLS + HD * (h + 1)],
                    idx16[:, h, :],
                    num_idxs=TOPK, num_idxs_reg=TOPK, elem_size=HD, elem_step=KVC,
                )
                nc.vector.tensor_copy(vselA[:, :, HD:HD + 1], onescol[:, :, None])
                nc.vector.tensor_copy(vselA[:, :, 0:HD], vsel)
                vselAs[h] = vselA

                kselT = psm.tile([128, TOPK], F32R, tag="kselT", bufs=4)
                if h2 == 0:
                    for kt in range(4):
                        tp = psP.tile([128, 128], F32, tag="PS")
                        tr(tp[:64, :], ksel[:, kt, :], ident)
                        nc.vector.tensor_copy(
                            kselT[:64, kt * 128:(kt + 1) * 128], tp[:64, :]
                        )
                else:
                    # matmul PSUM out must start at partition 0; build at
                    # base 0 then partition-shift via SBUF->SBUF DMA.
                    ktmp = psm.tile([64, TOPK], F32R, tag="ktmp", bufs=1)
                    for kt in range(4):
                        tp = psP.tile([128, 128], F32, tag="PS")
                        tr(tp[:64, :], ksel[:, kt, :], ident)
                        nc.vector.tensor_copy(
                            ktmp[:, kt * 128:(kt + 1) * 128], tp[:64, :]
                        )
                    nc.sync.dma_start(kselT[64:128, :], ktmp)
                kselTs[h] = kselT

            for qc in range(8):
                for h in range(HPC):
                    p2, h2 = h // 2, h % 2
                    base = 64 * h2
                    kselT, vselA = kselTs[h], vselAs[h]
                    avp = psP.tile([HD + 1, 512], F32, tag="P1")
                    for kp in range(2):
                        lp = psP.tile([128, 2, 512], F32, tag="P2x2")
                        for kj in range(2):
                            kt = 2 * kp + kj
                            mm(
                                lp[:, kj, :],
                                kselT[base:base + 64, kt * 128:(kt + 1) * 128],
                                qT[p2][base:base + 64, qc * 512:(qc + 1) * 512],
                                start=True, stop=True,
                            )
                        expT = pw.tile([128, 2, 512], F32R, tag="expT", bufs=2,
                                       name="expT")
                        nc.scalar.activation(
                            expT.rearrange("p a b -> p (a b)"),
                            lp.rearrange("p a b -> p (a b)"),
                            AF.Exp, scale=0.125,
                        )
                        for kj in range(2):
                            kt = 2 * kp + kj
                            mm(
                                avp, vselA[:, kt, :], expT[:, kj, :],
                                start=(kt == 0), stop=(kt == 3),
                            )
                    av = psm.tile([HD + 1, 512], F32, tag="av", bufs=2)
                    nc.vector.tensor_copy(av, avp)
                    for qs in range(4):
                        qt = qc * 4 + qs
                        tp2 = psP.tile([128, HD + 1], F32, tag="PS")
                        tr(
                            tp2, av[:, qs * 128:(qs + 1) * 128],
                            ident[:HD + 1, :HD + 1]
                        )
                        rcp = psm.tile([128, 1], F32, tag="rcp")
                        nc.vector.reciprocal(rcp, tp2[:, HD:HD + 1])
                        ob = obuf[qt // 16]
                        nc.vector.tensor_scalar_mul(
                            ob[:, qt % 16, HD * h:HD * (h + 1)], tp2[:, :HD], rcp
                        )

            # ---------- stage D: output ----------
            for q4 in range(8):
                qt = q4 * 4
                nc.sync.dma_start(
                    out[qt * 128:(qt + 4) * 128, :].rearrange(
                        "(q p) n -> p q n", p=128),
                    obuf[qt // 16][:, qt % 16:qt % 16 + 4, :],
                )

    nc.compile()
    return nc


_NC = None


def _get_nc():
    global _NC
    if _NC is None:
        _NC = build_nc()
    return _NC


def make_in_maps(**inputs):
    t = {k: np.ascontiguousarray(np.asarray(v, dtype=np.float32))
         for k, v in inputs.items()}
    wi1 = t["Wi1"]
    wi1h = wi1.astype(ml_dtypes.bfloat16)
    wi1l = (wi1 - wi1h.astype(np.float32)).astype(ml_dtypes.bfloat16)
    wi2 = t["Wi2"]
    wi2h = wi2.astype(ml_dtypes.bfloat16)
    wi2l = (wi2 - wi2h.astype(np.float32)).astype(ml_dtypes.bfloat16)
    in_maps = []
    for c in range(8):
        b, hg = c // 4, c % 4
        cs = COLS * hg
        hs = HPC * hg
        tok = t["tokens"][b]                       # [L, D]
        tokT = np.ascontiguousarray(tok.T)         # [D, L]
        mloc = tokT[:, hg * LLOC:(hg + 1) * LLOC]  # local quarter
        mth = mloc.astype(ml_dtypes.bfloat16)
        mtl = (mloc - mth.astype(np.float32)).astype(ml_dtypes.bfloat16)
        hr = np.array([16 * (p % 4) + hs + p // 4 for p in range(16)],
                      dtype=np.int16)[:, None]
        hr = np.tile(hr, (8, 1))
        in_maps.append({
            "tok_t": tokT,
            "mth_t": np.ascontiguousarray(mth),
            "mtl_t": np.ascontiguousarray(mtl),
            "wq": np.ascontiguousarray(t["Wq"][:, cs:cs + COLS]),
            "bq": np.ascontiguousarray(t["bq"][cs:cs + COLS]),
            "wkv": np.ascontiguousarray(
                np.concatenate([t["Wk"][:, cs:cs + COLS],
                                t["Wv"][:, cs:cs + COLS]], axis=1)),
            "bkv": np.ascontiguousarray(
                np.concatenate([t["bk"][cs:cs + COLS],
                                t["bv"][cs:cs + COLS]])[None, :]),
            "wi1h": np.ascontiguousarray(wi1h),
            "wi1l": np.ascontiguousarray(wi1l),
            "bi1": t["bi1"],
            "wi2h": np.ascontiguousarray(wi2h),
            "wi2l": np.ascontiguousarray(wi2l),
            "bi2": t["bi2"],
            "hrows": hr,
        })
    return in_maps


def kernel(**inputs) -> np.ndarray:
    nc = _get_nc()
    in_maps = make_in_maps(**inputs)
    res = run_bass_kernel_spmd(nc, in_maps, core_ids=list(range(8)))
    out = np.empty((B, L, D), dtype=np.float32)
    for c in range(8):
        b, hg = c // 4, c % 4
        out[b, :, COLS * hg:COLS * (hg + 1)] = res.results[c]["out"]
    return out
